# revision 32
# baseline (speedup 1.0000x reference)
"""Trainium2 Bass kernel for nn_ActorGraphPolicy (GNN message passing).

Data-parallel across 8 NeuronCores: each core handles 2048 of the 16384
batch rows. Feature-major on-chip layout (features on partitions, batch on
the free dim) so the tiny shared MLP weights are matmul-stationary.

Key design points:
  - Single activation-table set: only Tanh/Relu/Square/Identity run on the
    scalar engine, so there is exactly one ACT_TABLE_LOAD in the kernel.
    All rsqrt work (F.normalize) uses a quake seed + one Newton step.
  - Latency-critical chains (BU message scan, TD md scan) use an f32 quake
    on DVE with scalar_tensor_tensor fusion and the square on ACT - no
    GPSIMD hops (GPSIMD costs ~1.25us fixed per op).
  - The latency-tolerant BU "h" path (bu_a) uses a bf16 quake with its
    Newton multiplies on GPSIMD, and is emitted one full level-pair ahead
    of its consumer.
  - Block-diagonal stationary weights pack pairs of 64x64 matmuls into one
    128x128 matmul.
  - BU messages stay resident in SBUF; the TD phase reuses those tiles as
    its xm input, overwriting the bottom half with tanh(md). No DRAM
    round-trip.
  - TD defers the action-head matmuls (W2a, aW3) to after the md-normalize
    chain so the tensor engine has fill work at the level-join stall.
"""

import contextlib

import numpy as np

import concourse.bass as bass
import concourse.tile as tile
from concourse import bacc, mybir
from concourse.bass_utils import run_bass_kernel_spmd

F32 = mybir.dt.float32
BF16 = mybir.dt.bfloat16
AF = mybir.ActivationFunctionType
ALU = mybir.AluOpType
I16 = mybir.dt.int16
I32 = mybir.dt.int32

N_CORES = 8
B, L, S, MSG, HID = 16384, 20, 64, 64, 256
BC = B // N_CORES          # batch per core (2048)
NBLK = BC // 128           # 128-row batch blocks (16)
NPAIR = L // 2             # level pairs (10)
HALF = BC // 2             # BU half width (1024)
CH = 512                   # TD chunk width
NCH = BC // CH             # TD chunks (4)
MAGIC16 = 0x5F37           # bf16 quake-rsqrt seed constant
MAGIC32 = 0x5F3759E0       # f32 quake-rsqrt seed constant

WNAMES = [
    "uW1", "ub1", "uW2", "ub2", "uW3", "ub3",
    "aW1", "ab1", "aW2", "ab2", "aW3", "ab3",
    "mW1", "mb1", "mW2", "mb2", "mW3", "mb3",
]


def _build(nc: bass.Bass):
    state = nc.dram_tensor("state", [BC, L, S], F32, kind="ExternalInput")
    w = {n: nc.dram_tensor(n, shp, F32, kind="ExternalInput")
         for n, shp in [
             ("uW1", [S, 64]), ("ub1", [64]), ("uW2", [64 + MSG, 64]),
             ("ub2", [64]), ("uW3", [64, MSG]), ("ub3", [MSG]),
             ("aW1", [2 * MSG, HID]), ("ab1", [HID]), ("aW2", [HID, HID]),
             ("ab2", [HID]), ("aW3", [HID, 1]), ("ab3", [1]),
             ("mW1", [2 * MSG, HID]), ("mb1", [HID]), ("mW2", [HID, HID]),
             ("mb2", [HID]), ("mW3", [HID, MSG]), ("mb3", [MSG]),
         ]}
    out_ext = nc.dram_tensor("out", [BC, L], F32, kind="ExternalOutput")

    with tile.TileContext(nc) as tc:
        _emit(tc, nc, state, w, out_ext)
    return nc


def _emit(tc, nc, state, w, out_ext):
    ctx = contextlib.ExitStack()

    # ---------------- persistent SBUF pools ----------------
    pw = ctx.enter_context(tc.tile_pool(name="weights", bufs=1))
    pxbu = ctx.enter_context(tc.tile_pool(name="xbu", bufs=1))
    pact = ctx.enter_context(tc.tile_pool(name="act", bufs=1))

    # ---------------- weights / constants ----------------
    def blockdiag64(name):
        t = pw.tile([128, 128], BF16, tag=name + "bd", name=name + "bd")
        nc.gpsimd.memset(t[:, :], 0.0)
        ap = w[name].ap()
        nc.gpsimd.dma_start(t[0:64, 0:64], ap[:, :])
        nc.gpsimd.dma_start(t[64:128, 64:128], ap[:, :])
        return t

    def dupbias(name):
        t = pw.tile([128, 1], F32, tag=name, name=name)
        ap = w[name].ap()[:, None]
        nc.gpsimd.dma_start(t[0:64, :], ap[:, :])
        nc.gpsimd.dma_start(t[64:128, :], ap[:, :])
        return t

    uW1bd = blockdiag64("uW1")
    uW3bd = blockdiag64("uW3")
    uW2t = pw.tile([128, 64], BF16, tag="uW2")
    nc.gpsimd.dma_start(uW2t[:, :], w["uW2"].ap()[:, :])
    ub1d = dupbias("ub1")
    ub2d = dupbias("ub2")
    ub3d = dupbias("ub3")
    mb3d = dupbias("mb3")

    bdones = pw.tile([128, 128], BF16, tag="bdones")
    nc.gpsimd.memset(bdones[:, :], 1.0)
    nc.gpsimd.memset(bdones[0:64, 64:128], 0.0)
    nc.gpsimd.memset(bdones[64:128, 0:64], 0.0)

    # TD L1 weights with row halves swapped: TD xm tile is [md ; mu].
    def w1perm(name):
        t = pw.tile([128, HID], BF16, tag=name + "p", name=name + "p")
        ap = w[name].ap()
        nc.gpsimd.dma_start(t[0:64, :], ap[64:128, :])
        nc.gpsimd.dma_start(t[64:128, :], ap[0:64, :])
        return t

    aW1p = w1perm("aW1")
    mW1p = w1perm("mW1")

    def ksplit(name, cols):
        ts = []
        for kh in range(2):
            t = pw.tile([128, cols], BF16, tag=f"{name}k{kh}", name=f"{name}k{kh}")
            nc.gpsimd.dma_start(t[:, :], w[name].ap()[kh * 128:(kh + 1) * 128, :])
            ts.append(t)
        return ts

    aW2k = ksplit("aW2", HID)
    mW2k = ksplit("mW2", HID)
    mW3k = ksplit("mW3", MSG)
    aW3k = ksplit("aW3", 1)

    def hbias(name):
        t0 = pw.tile([128, 1], F32, tag=name + "0", name=name + "0")
        t1 = pw.tile([128, 1], F32, tag=name + "1", name=name + "1")
        ap = w[name].ap()[:, None]
        nc.gpsimd.dma_start(t0[:, :], ap[0:128, :])
        nc.gpsimd.dma_start(t1[:, :], ap[128:256, :])
        return t0, t1

    ab1t = hbias("ab1")
    ab2t = hbias("ab2")
    mb1t = hbias("mb1")
    mb2t = hbias("mb2")
    ab3t = pw.tile([32, 1], F32, tag="ab3")
    nc.gpsimd.dma_start(ab3t[0:1, :], w["ab3"].ap()[:, None])
    nc.gpsimd.partition_broadcast(ab3t[:, :], ab3t[0:1, :], channels=32)

    ident = pw.tile([128, 128], BF16, tag="ident")
    from concourse.masks import make_identity
    make_identity(nc, ident[:, :])

    a_store = pact.tile([32, BC], F32, tag="a_store")

    # per-level xm tiles, SBUF-resident for the whole kernel:
    #   BU: xbu[l] = [tanh(h_l) ; tanh(m_{l+1})]
    #   TD: xbu[l-1] reused as X_l = [tanh(md_{l-1}) ; tanh(mu_l)]
    xbu = {}

    def get_xbu(l):
        if l not in xbu:
            xbu[l] = pxbu.tile([128, BC], BF16, tag=f"xbu{l}", name=f"xbu{l}")
        return xbu[l]

    # f32 quake rsqrt for latency chains: y1 ~= rsqrt(s), s a PSUM f32 tile.
    # seed on DVE straight from PSUM bits, square on ACT, Newton fused with
    # scalar_tensor_tensor. 4 DVE ops + 1 ACT op, no copies, no GPSIMD.
    def quake32(pool, nsb, W, tg):
        t = pool.tile([128, W], F32, tag=tg + "t", name=tg + "t")
        nc.vector.tensor_scalar(
            t[:, :].bitcast(I32), nsb.bitcast(I32), 1, -1,
            op0=ALU.arith_shift_right, op1=ALU.bitwise_xor)
        y0 = pool.tile([128, W], F32, tag=tg + "y0", name=tg + "y0")
        nc.vector.tensor_scalar_add(y0[:, :].bitcast(I32),
                                    t[:, :].bitcast(I32), MAGIC32)
        wt = pool.tile([128, W], F32, tag=tg + "w", name=tg + "w")
        nc.scalar.activation(wt[:, :], y0[:, :], AF.Square)
        u = pool.tile([128, W], F32, tag=tg + "u", name=tg + "u")
        nc.vector.scalar_tensor_tensor(u[:, :], wt[:, :], -0.5, nsb,
                                       op0=ALU.mult, op1=ALU.mult)
        y1 = pool.tile([128, W], F32, tag=tg + "y1", name=tg + "y1")
        nc.vector.scalar_tensor_tensor(y1[:, :], u[:, :], 1.5, y0[:, :],
                                       op0=ALU.add, op1=ALU.mult)
        return y1

    # ---------------- BU phase ----------------
    bu_ctx = contextlib.ExitStack()
    pbw = bu_ctx.enter_context(tc.tile_pool(name="buwork", bufs=2))
    pbh = bu_ctx.enter_context(tc.tile_pool(name="buhalf", bufs=2))
    ppA = bu_ctx.enter_context(tc.tile_pool(name="psA", bufs=1, space="PSUM"))
    ppB = bu_ctx.enter_context(tc.tile_pool(name="psB", bufs=1, space="PSUM"))

    def emit_bu_a_front(p, xts, g, hbw, sqbw):
        """bu_a part 1 for group g of pair p: matmuls + PSUM drains into the
        pair-wide hbw/sqbw tiles [128, 2048]."""
        c0 = g * HALF
        gh = slice(c0, c0 + HALF)
        ha = ppA.tile([128, HALF], F32, tag="pa", name="ha", bufs=2)
        for j in range(2):
            jj = slice(j * 512, (j + 1) * 512)
            cj = slice(c0 + j * 512, c0 + (j + 1) * 512)
            nc.tensor.matmul(ha[:, jj], uW1bd[:, :], xts[p][:, cj])
        nc.scalar.activation(hbw[:, gh], ha[:, :], AF.Identity,
                             bias=ub1d[:, 0:1])
        sq = pbw.tile([128, HALF], BF16, tag="sqa", name="sqa")
        nc.gpsimd.tensor_mul(sq[:, :], hbw[:, gh], hbw[:, gh])
        nsq = ppA.tile([128, HALF], F32, tag="pa", name="nsq", bufs=2)
        for j in range(2):
            jj = slice(j * 512, (j + 1) * 512)
            nc.tensor.matmul(nsq[:, jj], bdones[:, :], sq[:, jj])
        nc.scalar.copy(sqbw[:, gh], nsq[:, :])

    def emit_bu_a_back(p, hbw, sqbw):
        """bu_a part 2, pair-wide [128, 2048]: bf16 quake + apply + tanh +
        unpack. Emitted after the bu_b chains of the current iteration so it
        never blocks them; all on DVE/ACT at 2x/4x bf16 rates."""
        l0, l1 = 2 * p, 2 * p + 1
        t = pbw.tile([128, BC], BF16, tag="qt", name="qt", bufs=1)
        nc.vector.tensor_scalar(
            t[:, :].bitcast(I32), sqbw[:, :].bitcast(I32), 1, 0x7FFF7FFF,
            op0=ALU.arith_shift_right, op1=ALU.bitwise_and)
        tn = pbw.tile([128, BC], BF16, tag="qtn", name="qtn", bufs=1)
        nc.vector.tensor_scalar(
            tn[:, :].bitcast(I16), t[:, :].bitcast(I16), -1, None,
            op0=ALU.bitwise_xor)
        y0 = pbw.tile([128, BC], BF16, tag="qy0", name="qy0", bufs=1)
        nc.vector.tensor_scalar_add(y0[:, :].bitcast(I16),
                                    tn[:, :].bitcast(I16), MAGIC16)
        wt = pbw.tile([128, BC], BF16, tag="qw", name="qw", bufs=1)
        nc.vector.tensor_mul(wt[:, :], y0[:, :], y0[:, :])
        u2 = pbw.tile([128, BC], BF16, tag="qu2", name="qu2", bufs=1)
        nc.vector.tensor_mul(u2[:, :], wt[:, :], sqbw[:, :])
        v = pbw.tile([128, BC], BF16, tag="qv", name="qv", bufs=1)
        nc.vector.tensor_scalar(v[:, :], u2[:, :], -0.5, 1.5,
                                op0=ALU.mult, op1=ALU.add)
        y1 = pbw.tile([128, BC], BF16, tag="qy1", name="qy1", bufs=1)
        nc.vector.tensor_mul(y1[:, :], v[:, :], y0[:, :])
        xaw = pbw.tile([128, BC], BF16, tag="xaw", name="xaw", bufs=1)
        nc.vector.tensor_mul(xaw[:, :], hbw[:, :], y1[:, :])
        txa = pbw.tile([128, BC], BF16, tag="txa", name="txa", bufs=1)
        nc.scalar.activation(txa[:, :], xaw[:, :], AF.Tanh)
        nc.vector.tensor_copy(get_xbu(l0)[0:64, :], txa[0:64, :])
        nc.vector.tensor_copy(get_xbu(l1)[0:64, :], txa[64:128, :])

    def emit_bu_b_half(l, g):
        """One level-step of the message chain for batch-half g (cols
        [g*1024, g*1024+1024), packed [128,512]). The two halves run as
        independent chains one level apart, so each op has a full
        iteration of slack."""
        X = get_xbu(l)
        Xn = get_xbu(l - 1)
        c0 = g * HALF
        h2p = ppB.tile([128, 512], F32, tag=f"bA{g}", name="h2p", bufs=1)
        nc.tensor.matmul(h2p[0:64, :], uW2t[:, :], X[:, c0:c0 + 512])
        nc.tensor.matmul(h2p[64:128, :], uW2t[:, :], X[:, c0 + 512:c0 + 1024])
        h2s = pbh.tile([128, 512], BF16, tag=f"h2s{g}", name="h2s", bufs=1)
        nc.scalar.activation(h2s[:, :], h2p[:, :], AF.Tanh, bias=ub2d[:, 0:1])
        msgp = ppB.tile([128, 512], F32, tag=f"bB{g}", name="msgp", bufs=1)
        nc.tensor.matmul(msgp[:, :], uW3bd[:, :], h2s[:, :])
        sqm = pbh.tile([128, 512], BF16, tag=f"sqm{g}", name="sqm", bufs=1)
        nc.scalar.activation(sqm[:, :], msgp[:, :], AF.Square,
                             bias=ub3d[:, 0:1])
        nsb = ppB.tile([128, 512], F32, tag=f"bA{g}", name="nsb", bufs=1)
        nc.tensor.matmul(nsb[:, :], bdones[:, :], sqm[:, :])
        tq = pbh.tile([128, 512], F32, tag=f"qbt{g}", name="qbt", bufs=1)
        nc.vector.tensor_scalar(
            tq[:, :].bitcast(I32), nsb[:, :].bitcast(I32), 1, -1,
            op0=ALU.arith_shift_right, op1=ALU.bitwise_xor)
        y0q = pbh.tile([128, 512], F32, tag=f"qby0{g}", name="qby0", bufs=1)
        nc.vector.tensor_scalar_add(y0q[:, :].bitcast(I32),
                                    tq[:, :].bitcast(I32), MAGIC32)
        wq = pbh.tile([128, 512], F32, tag=f"qbw{g}", name="qbw", bufs=1)
        nc.scalar.activation(wq[:, :], y0q[:, :], AF.Square)
        uq = pbh.tile([128, 512], F32, tag=f"qbu{g}", name="qbu", bufs=1)
        nc.vector.scalar_tensor_tensor(uq[:, :], wq[:, :], -0.5, nsb[:, :],
                                       op0=ALU.mult, op1=ALU.mult)
        y1q = pbh.tile([128, 512], F32, tag=f"qby1{g}", name="qby1", bufs=1)
        nc.vector.scalar_tensor_tensor(y1q[:, :], uq[:, :], 1.5, y0q[:, :],
                                       op0=ALU.add, op1=ALU.mult)
        tms = pbh.tile([128, 512], BF16, tag=f"tms{g}", name="tms", bufs=1)
        nc.vector.scalar_tensor_tensor(tms[:, :], msgp[:, :], ub3d[:, 0:1],
                                       y1q[:, :], op0=ALU.add, op1=ALU.mult)
        nc.scalar.activation(Xn[64:128, c0:c0 + 512], tms[0:64, :], AF.Tanh)
        nc.scalar.activation(Xn[64:128, c0 + 512:c0 + 1024], tms[64:128, :],
                             AF.Tanh)

    # state view: [pair, partition(batch%128), block, 2*S contiguous values]
    st_pair = state.ap().rearrange("(k p) (lp w) v -> lp p k (w v)", p=128, w=2)

    with tc.tile_pool(name="xtpool", bufs=3) as pxt:

        def make_xt(p):
            xt = pxt.tile([128, BC], BF16, tag="xt", name=f"xt{p}")
            for kg in range(2):
                stg = pxt.tile([128, 8 * 2 * S], BF16, tag="stg", name="stg",
                               bufs=2)
                nc.gpsimd.dma_start(
                    stg[:, :].rearrange("q (k u) -> q k u", k=8),
                    st_pair[p][:, 8 * kg:8 * (kg + 1)])
                tp = ppB.tile([128, 1024], BF16, tag=f"bA{kg}", name="tp",
                              bufs=1)
                for ki in range(8):
                    nc.tensor.transpose(tp[:, ki * 128:(ki + 1) * 128],
                                        stg[:, ki * 128:(ki + 1) * 128],
                                        ident[:, :])
                nc.scalar.copy(xt[:, kg * 1024:(kg + 1) * 1024], tp[:, :])
            return xt

        xts = {NPAIR - 1: make_xt(NPAIR - 1), NPAIR - 2: make_xt(NPAIR - 2)}
        nc.gpsimd.memset(get_xbu(L - 1)[64:128, :], 0.0)  # tanh(m(20)) = 0

        def alloc_ab():
            hbw = pbw.tile([128, BC], BF16, tag="hbw", name="hbw", bufs=2)
            sqbw = pbw.tile([128, BC], BF16, tag="sqbw", name="sqbw", bufs=2)
            return hbw, sqbw

        hs = alloc_ab()
        for g in range(2):
            emit_bu_a_front(NPAIR - 1, xts, g, *hs)
        emit_bu_a_back(NPAIR - 1, *hs)
        del xts[NPAIR - 1]
        # chain g=0 runs levels 19..0; chain g=1 lags one level behind.
        hs_pend = None
        for l0 in range(L - 1, -1, -1):
            if l0 + 1 <= L - 1:
                emit_bu_b_half(l0 + 1, 1)
            if l0 % 2 == 1:
                pf = (l0 - 1) // 2 - 1
                if pf >= 0:
                    hs_pend = alloc_ab()
                    for g in range(2):
                        emit_bu_a_front(pf, xts, g, *hs_pend)
                if pf >= 1:
                    xts[pf - 1] = make_xt(pf - 1)
            emit_bu_b_half(l0, 0)
            if l0 % 2 == 0 and l0 >= 2:
                p = l0 // 2 - 1
                emit_bu_a_back(p, *hs_pend)
                del xts[p]
        emit_bu_b_half(0, 1)

    bu_ctx.close()

    # ---------------- TD phase ----------------
    td_ctx = contextlib.ExitStack()
    ptd = td_ctx.enter_context(tc.tile_pool(name="tdwork", bufs=2))
    pmd = td_ctx.enter_context(tc.tile_pool(name="mdwork", bufs=2))
    ppL = {}
    for mlp in "am":
        for mh in range(2):
            ppL[(mlp, mh)] = td_ctx.enter_context(
                tc.tile_pool(name=f"psL{mlp}{mh}", bufs=1, space="PSUM"))
    ppMD = td_ctx.enter_context(tc.tile_pool(name="psMD", bufs=1, space="PSUM"))
    ppNS = td_ctx.enter_context(tc.tile_pool(name="psNS", bufs=1, space="PSUM"))
    ppT = td_ctx.enter_context(tc.tile_pool(name="psT", bufs=1, space="PSUM"))

    nc.gpsimd.memset(get_xbu(-1)[0:64, :], 0.0)  # tanh(md(-1)) = 0

    def relu(ps, hs, bias, on_act):
        if on_act:
            nc.scalar.activation(hs[:, :], ps[:, :], AF.Relu, bias=bias[:, 0:1])
        else:
            nc.vector.tensor_scalar(hs[:, :], ps[:, :], bias[:, 0:1], 0.0,
                                    op0=ALU.add, op1=ALU.max)

    for l in range(L):
        X = get_xbu(l - 1)
        last = (l == L - 1)
        aps = ppT.tile([97, 512], F32, tag="aps")
        h1a = {}
        h2a = {}
        # pass 1 per chunk: W1 (a+m) + relus, W2m + relus, mW3 -> packed mdg
        # mdg [128,1024]: even chunks on partitions 0:64, odd on 64:128;
        # chunk pair (2h, 2h+1) in columns h*512:(h+1)*512.
        if not last:
            mdg = ppMD.tile([128, 1024], F32, tag="md", name="mdg")
        for c in range(NCH):
            cc = slice(c * CH, (c + 1) * CH)
            mlps = "a" if last else "am"
            h1 = {}
            for mlp in mlps:
                W1p = aW1p if mlp == "a" else mW1p
                for mh in range(2):
                    ps = ppL[(mlp, mh)].tile([128, CH], F32, tag=f"L{mlp}{mh}",
                                             name=f"L{mlp}{mh}")
                    nc.tensor.matmul(ps[:, :], W1p[:, mh * 128:(mh + 1) * 128],
                                     X[:, cc])
                    tag = f"h1a{mh}c{c}" if mlp == "a" else f"h1m{mh}"
                    hs = ptd.tile([128, CH], BF16, tag=tag, name=tag)
                    relu(ps, hs, (ab1t if mlp == "a" else mb1t)[mh],
                         on_act=(mlp == "a"))
                    h1[(mlp, mh)] = hs
            h1a[c] = (h1[("a", 0)], h1[("a", 1)])
            if last:
                continue
            # W2m + relu + mW3 (the latency-critical md branch)
            h2m = {}
            for mh in range(2):
                ps = ppL[("m", mh)].tile([128, CH], F32, tag=f"Lm{mh}",
                                         name=f"L2m{mh}")
                ms_ = slice(mh * 128, (mh + 1) * 128)
                nc.tensor.matmul(ps[:, :], mW2k[0][:, ms_], h1[("m", 0)][:, :],
                                 start=True, stop=False)
                nc.tensor.matmul(ps[:, :], mW2k[1][:, ms_], h1[("m", 1)][:, :],
                                 start=False, stop=True)
                hs = ptd.tile([128, CH], BF16, tag=f"h2m{mh}", name=f"h2m{mh}")
                relu(ps, hs, mb2t[mh], on_act=False)
                h2m[mh] = hs
            pp = slice(64 * (c % 2), 64 * (c % 2) + 64)
            ccol = slice((c // 2) * 512, (c // 2) * 512 + 512)
            tpos = (0, 64 * (c % 2))
            nc.tensor.matmul(mdg[pp, ccol], mW3k[0][:, :], h2m[0][:, :],
                             start=True, stop=False, tile_position=tpos)
            nc.tensor.matmul(mdg[pp, ccol], mW3k[1][:, :], h2m[1][:, :],
                             start=False, stop=True, tile_position=tpos)
        if not last:
            # md normalize chain (bias-add + square emitted now; the sum
            # matmuls after the W2a fill work below).
            mdbw = pmd.tile([128, 1024], BF16, tag="mdbw", name="mdbw")
            nc.scalar.activation(mdbw[:, :], mdg[:, :], AF.Identity,
                                 bias=mb3d[:, 0:1])
            sqd = pmd.tile([128, 1024], BF16, tag="sqd", name="sqd")
            nc.gpsimd.tensor_mul(sqd[:, :], mdbw[:, :], mdbw[:, :])
        # W2a + relu: PE fill work at the level join, split around the
        # nsq matmuls so the tensor engine stays fed during the quake tail.
        def w2a_chunk(c):
            for mh in range(2):
                ps = ppL[("a", mh)].tile([128, CH], F32, tag=f"La{mh}",
                                         name=f"L2a{mh}")
                ms_ = slice(mh * 128, (mh + 1) * 128)
                nc.tensor.matmul(ps[:, :], aW2k[0][:, ms_],
                                 h1a[c][0][:, :], start=True, stop=False)
                nc.tensor.matmul(ps[:, :], aW2k[1][:, ms_],
                                 h1a[c][1][:, :], start=False, stop=True)
                hs = ptd.tile([128, CH], BF16, tag=f"h2a{mh}c{c}",
                              name=f"h2a{mh}c{c}")
                relu(ps, hs, ab2t[mh], on_act=True)
                h2a[c] = h2a.get(c, {})
                h2a[c][mh] = hs

        w2a_chunk(0)
        w2a_chunk(1)
        Xn1 = get_xbu(l)

        def md_half(h):
            hh = slice(h * 512, (h + 1) * 512)
            nsq = ppNS.tile([128, 512], F32, tag="nsq", name="nsq")
            nc.tensor.matmul(nsq[:, :], bdones[:, :], sqd[:, hh])
            y1 = quake32(pmd, nsq[:, :], 512, "qd")
            tms = pmd.tile([128, 512], BF16, tag="tmsd", name="tmsd")
            nc.vector.tensor_mul(tms[:, :], mdbw[:, hh], y1[:, :])
            ce, co = 2 * h * CH, (2 * h + 1) * CH
            nc.scalar.activation(Xn1[0:64, ce:ce + CH], tms[0:64, :],
                                 AF.Tanh)
            nc.scalar.activation(Xn1[0:64, co:co + CH], tms[64:128, :],
                                 AF.Tanh)

        if not last:
            md_half(0)
        w2a_chunk(2)
        if not last:
            md_half(1)
        w2a_chunk(3)
        # aW3 (more PE fill), then store the action row
        for c in range(NCH):
            nc.tensor.matmul(aps[32 * c:32 * c + 1, :], aW3k[0][:, :],
                             h2a[c][0][:, :], start=True, stop=False,
                             tile_position=(0, 32 * c))
            nc.tensor.matmul(aps[32 * c:32 * c + 1, :], aW3k[1][:, :],
                             h2a[c][1][:, :], start=False, stop=True,
                             tile_position=(0, 32 * c))
        asb = ptd.tile([97, 512], F32, tag="asb")
        nc.vector.tensor_copy(asb[:, :], aps[:, :])
        nc.scalar.dma_start(a_store[l:l + 1, :], asb[0:97:32, :])

    td_ctx.close()

    # ---------------- output: tanh, transpose, DMA ----------------
    pout = ctx.enter_context(tc.tile_pool(name="outp", bufs=1))
    att = pout.tile([32, BC], F32, tag="att")
    nc.gpsimd.memset(att[:, :], 0.0)
    nc.scalar.activation(att[0:20, :], a_store[0:20, :], AF.Tanh,
                         bias=ab3t[0:20, 0:1])
    otr = pout.tile([32, BC], F32, tag="otr")
    for k in range(NBLK):
        nc.vector.transpose(otr[:, k * 128:(k + 1) * 128],
                            att[:, k * 128:(k + 1) * 128])
    # otr[r, k*128 + 32*bj + c] = action(batch k*128 + 32*bj + r, level c)
    dst = out_ext.ap().rearrange("(k bj r) l -> r k bj l", r=32, bj=4)
    src = otr[:, :].rearrange("r (k bj c) -> r k bj c", bj=4, c=32)[:, :, :, 0:20]
    nc.sync.dma_start(dst, src)

    ctx.close()


_NC_CACHE = None


def _get_nc():
    global _NC_CACHE
    if _NC_CACHE is None:
        nc = bacc.Bacc("TRN2", target_bir_lowering=False, debug=False)
        _build(nc)
        nc.compile()
        _NC_CACHE = nc
    return _NC_CACHE


def kernel(**inputs) -> np.ndarray:
    nc = _get_nc()
    state = inputs["state"]
    in_maps = []
    for i in range(N_CORES):
        m = {"state": np.ascontiguousarray(state[i * BC:(i + 1) * BC])}
        for n in WNAMES:
            m[n] = np.ascontiguousarray(inputs[n])
        in_maps.append(m)
    res = run_bass_kernel_spmd(nc, in_maps, core_ids=list(range(N_CORES)))
    return np.concatenate([res.results[i]["out"] for i in range(N_CORES)], axis=0)


# revision 34
# speedup vs baseline: 1.1242x; 1.1242x over previous
"""Trainium2 Bass kernel for nn_ActorGraphPolicy (GNN message passing).

Data-parallel across 8 NeuronCores: each core handles 2048 of the 16384
batch rows. Feature-major on-chip layout (features on partitions, batch on
the free dim) so the tiny shared MLP weights are matmul-stationary.

Key design points:
  - Single activation-table set: only Tanh/Relu/Square/Identity run on the
    scalar engine, so there is exactly one ACT_TABLE_LOAD in the kernel.
    All rsqrt work (F.normalize) uses a quake seed + one Newton step.
  - Latency-critical chains (BU message scan, TD md scan) use an f32 quake
    on DVE with scalar_tensor_tensor fusion and the square on ACT - no
    GPSIMD hops (GPSIMD costs ~1.25us fixed per op).
  - The latency-tolerant BU "h" path (bu_a) uses a bf16 quake with its
    Newton multiplies on GPSIMD, and is emitted one full level-pair ahead
    of its consumer.
  - Block-diagonal stationary weights pack pairs of 64x64 matmuls into one
    128x128 matmul.
  - BU messages stay resident in SBUF; the TD phase reuses those tiles as
    its xm input, overwriting the bottom half with tanh(md). No DRAM
    round-trip.
  - TD defers the action-head matmuls (W2a, aW3) to after the md-normalize
    chain so the tensor engine has fill work at the level-join stall.
"""

import contextlib

import numpy as np

import concourse.bass as bass
import concourse.tile as tile
from concourse import bacc, mybir
from concourse.bass_utils import run_bass_kernel_spmd

F32 = mybir.dt.float32
BF16 = mybir.dt.bfloat16
AF = mybir.ActivationFunctionType
ALU = mybir.AluOpType
I16 = mybir.dt.int16
I32 = mybir.dt.int32

N_CORES = 8
B, L, S, MSG, HID = 16384, 20, 64, 64, 256
BC = B // N_CORES          # batch per core (2048)
NBLK = BC // 128           # 128-row batch blocks (16)
NPAIR = L // 2             # level pairs (10)
HALF = BC // 2             # BU half width (1024)
CH = 512                   # TD chunk width
NCH = BC // CH             # TD chunks (4)
MAGIC16 = 0x5F37           # bf16 quake-rsqrt seed constant
MAGIC32 = 0x5F3759E0       # f32 quake-rsqrt seed constant

WNAMES = [
    "uW1", "ub1", "uW2", "ub2", "uW3", "ub3",
    "aW1", "ab1", "aW2", "ab2", "aW3", "ab3",
    "mW1", "mb1", "mW2", "mb2", "mW3", "mb3",
]


def _build(nc: bass.Bass):
    state = nc.dram_tensor("state", [BC, L, S], F32, kind="ExternalInput")
    w = {n: nc.dram_tensor(n, shp, F32, kind="ExternalInput")
         for n, shp in [
             ("uW1", [S, 64]), ("ub1", [64]), ("uW2", [64 + MSG, 64]),
             ("ub2", [64]), ("uW3", [64, MSG]), ("ub3", [MSG]),
             ("aW1", [2 * MSG, HID]), ("ab1", [HID]), ("aW2", [HID, HID]),
             ("ab2", [HID]), ("aW3", [HID, 1]), ("ab3", [1]),
             ("mW1", [2 * MSG, HID]), ("mb1", [HID]), ("mW2", [HID, HID]),
             ("mb2", [HID]), ("mW3", [HID, MSG]), ("mb3", [MSG]),
         ]}
    out_ext = nc.dram_tensor("out", [BC, L], F32, kind="ExternalOutput")

    with tile.TileContext(nc) as tc:
        _emit(tc, nc, state, w, out_ext)
    return nc


def _emit(tc, nc, state, w, out_ext):
    ctx = contextlib.ExitStack()

    # ---------------- persistent SBUF pools ----------------
    pw = ctx.enter_context(tc.tile_pool(name="weights", bufs=1))
    pxbu = ctx.enter_context(tc.tile_pool(name="xbu", bufs=1))
    pact = ctx.enter_context(tc.tile_pool(name="act", bufs=1))

    # ---------------- weights / constants ----------------
    def blockdiag64(name):
        t = pw.tile([128, 128], BF16, tag=name + "bd", name=name + "bd")
        nc.gpsimd.memset(t[:, :], 0.0)
        ap = w[name].ap()
        nc.gpsimd.dma_start(t[0:64, 0:64], ap[:, :])
        nc.gpsimd.dma_start(t[64:128, 64:128], ap[:, :])
        return t

    def dupbias(name):
        t = pw.tile([128, 1], F32, tag=name, name=name)
        ap = w[name].ap()[:, None]
        nc.gpsimd.dma_start(t[0:64, :], ap[:, :])
        nc.gpsimd.dma_start(t[64:128, :], ap[:, :])
        return t

    uW1bd = blockdiag64("uW1")
    uW3bd = blockdiag64("uW3")
    uW2t = pw.tile([128, 64], BF16, tag="uW2")
    nc.gpsimd.dma_start(uW2t[:, :], w["uW2"].ap()[:, :])
    ub1d = dupbias("ub1")
    ub2d = dupbias("ub2")
    ub3d = dupbias("ub3")
    mb3d = dupbias("mb3")

    bdones = pw.tile([128, 128], BF16, tag="bdones")
    nc.gpsimd.memset(bdones[:, :], 1.0)
    nc.gpsimd.memset(bdones[0:64, 64:128], 0.0)
    nc.gpsimd.memset(bdones[64:128, 0:64], 0.0)

    # TD L1 weights with row halves swapped: TD xm tile is [md ; mu].
    def w1perm(name):
        t = pw.tile([128, HID], BF16, tag=name + "p", name=name + "p")
        ap = w[name].ap()
        nc.gpsimd.dma_start(t[0:64, :], ap[64:128, :])
        nc.gpsimd.dma_start(t[64:128, :], ap[0:64, :])
        return t

    aW1p = w1perm("aW1")
    mW1p = w1perm("mW1")

    def ksplit(name, cols):
        ts = []
        for kh in range(2):
            t = pw.tile([128, cols], BF16, tag=f"{name}k{kh}", name=f"{name}k{kh}")
            nc.gpsimd.dma_start(t[:, :], w[name].ap()[kh * 128:(kh + 1) * 128, :])
            ts.append(t)
        return ts

    aW2k = ksplit("aW2", HID)
    mW2k = ksplit("mW2", HID)
    mW3k = ksplit("mW3", MSG)
    aW3k = ksplit("aW3", 1)

    def hbias(name):
        t0 = pw.tile([128, 1], F32, tag=name + "0", name=name + "0")
        t1 = pw.tile([128, 1], F32, tag=name + "1", name=name + "1")
        ap = w[name].ap()[:, None]
        nc.gpsimd.dma_start(t0[:, :], ap[0:128, :])
        nc.gpsimd.dma_start(t1[:, :], ap[128:256, :])
        return t0, t1

    ab1t = hbias("ab1")
    ab2t = hbias("ab2")
    mb1t = hbias("mb1")
    mb2t = hbias("mb2")
    ab3t = pw.tile([32, 1], F32, tag="ab3")
    nc.gpsimd.dma_start(ab3t[0:1, :], w["ab3"].ap()[:, None])
    nc.gpsimd.partition_broadcast(ab3t[:, :], ab3t[0:1, :], channels=32)

    ident = pw.tile([128, 128], BF16, tag="ident")
    from concourse.masks import make_identity
    make_identity(nc, ident[:, :])

    a_store = pact.tile([32, BC], F32, tag="a_store")

    # per-level xm tiles, SBUF-resident for the whole kernel:
    #   BU: xbu[l] = [tanh(h_l) ; tanh(m_{l+1})]
    #   TD: xbu[l-1] reused as X_l = [tanh(md_{l-1}) ; tanh(mu_l)]
    xbu = {}

    def get_xbu(l):
        if l not in xbu:
            xbu[l] = pxbu.tile([128, BC], BF16, tag=f"xbu{l}", name=f"xbu{l}")
        return xbu[l]

    # f32 quake rsqrt for latency chains: y1 ~= rsqrt(s), s a PSUM f32 tile.
    # seed on DVE straight from PSUM bits, square on ACT, Newton fused with
    # scalar_tensor_tensor. 4 DVE ops + 1 ACT op, no copies, no GPSIMD.
    def quake32(pool, nsb, W, tg):
        t = pool.tile([128, W], F32, tag=tg + "t", name=tg + "t")
        nc.vector.tensor_scalar(
            t[:, :].bitcast(I32), nsb.bitcast(I32), 1, -1,
            op0=ALU.arith_shift_right, op1=ALU.bitwise_xor)
        y0 = pool.tile([128, W], F32, tag=tg + "y0", name=tg + "y0")
        nc.vector.tensor_scalar_add(y0[:, :].bitcast(I32),
                                    t[:, :].bitcast(I32), MAGIC32)
        wt = pool.tile([128, W], F32, tag=tg + "w", name=tg + "w")
        nc.scalar.activation(wt[:, :], y0[:, :], AF.Square)
        u = pool.tile([128, W], F32, tag=tg + "u", name=tg + "u")
        nc.vector.scalar_tensor_tensor(u[:, :], wt[:, :], -0.5, nsb,
                                       op0=ALU.mult, op1=ALU.mult)
        y1 = pool.tile([128, W], F32, tag=tg + "y1", name=tg + "y1")
        nc.vector.scalar_tensor_tensor(y1[:, :], u[:, :], 1.5, y0[:, :],
                                       op0=ALU.add, op1=ALU.mult)
        return y1

    # ---------------- BU phase ----------------
    bu_ctx = contextlib.ExitStack()
    pbw = bu_ctx.enter_context(tc.tile_pool(name="buwork", bufs=2))
    pbh = bu_ctx.enter_context(tc.tile_pool(name="buhalf", bufs=2))
    ppA = bu_ctx.enter_context(tc.tile_pool(name="psA", bufs=1, space="PSUM"))
    ppB = bu_ctx.enter_context(tc.tile_pool(name="psB", bufs=1, space="PSUM"))

    def emit_bu_a_front(p, xts, g, hbw, sqbw):
        """bu_a part 1 for group g of pair p: matmuls + PSUM drains into the
        pair-wide hbw/sqbw tiles [128, 2048]."""
        c0 = g * HALF
        gh = slice(c0, c0 + HALF)
        ha = ppA.tile([128, HALF], F32, tag="pa", name="ha", bufs=2)
        for j in range(2):
            jj = slice(j * 512, (j + 1) * 512)
            cj = slice(c0 + j * 512, c0 + (j + 1) * 512)
            nc.tensor.matmul(ha[:, jj], uW1bd[:, :], xts[p][:, cj])
        nc.scalar.activation(hbw[:, gh], ha[:, :], AF.Identity,
                             bias=ub1d[:, 0:1])
        sq = pbw.tile([128, HALF], BF16, tag="sqa", name="sqa")
        nc.gpsimd.tensor_mul(sq[:, :], hbw[:, gh], hbw[:, gh])
        nsq = ppA.tile([128, HALF], F32, tag="pa", name="nsq", bufs=2)
        for j in range(2):
            jj = slice(j * 512, (j + 1) * 512)
            nc.tensor.matmul(nsq[:, jj], bdones[:, :], sq[:, jj])
        nc.scalar.copy(sqbw[:, gh], nsq[:, :])

    def emit_bu_a_back(p, hbw, sqbw):
        """bu_a part 2, pair-wide [128, 2048]: bf16 quake + apply + tanh +
        unpack. Emitted after the bu_b chains of the current iteration so it
        never blocks them; all on DVE/ACT at 2x/4x bf16 rates."""
        l0, l1 = 2 * p, 2 * p + 1
        t = pbw.tile([128, BC], BF16, tag="qt", name="qt", bufs=1)
        nc.vector.tensor_scalar(
            t[:, :].bitcast(I32), sqbw[:, :].bitcast(I32), 1, 0x7FFF7FFF,
            op0=ALU.arith_shift_right, op1=ALU.bitwise_and)
        tn = pbw.tile([128, BC], BF16, tag="qtn", name="qtn", bufs=1)
        nc.vector.tensor_scalar(
            tn[:, :].bitcast(I16), t[:, :].bitcast(I16), -1, None,
            op0=ALU.bitwise_xor)
        y0 = pbw.tile([128, BC], BF16, tag="qy0", name="qy0", bufs=1)
        nc.vector.tensor_scalar_add(y0[:, :].bitcast(I16),
                                    tn[:, :].bitcast(I16), MAGIC16)
        wt = pbw.tile([128, BC], BF16, tag="qw", name="qw", bufs=1)
        nc.vector.tensor_mul(wt[:, :], y0[:, :], y0[:, :])
        u2 = pbw.tile([128, BC], BF16, tag="qu2", name="qu2", bufs=1)
        nc.vector.tensor_mul(u2[:, :], wt[:, :], sqbw[:, :])
        v = pbw.tile([128, BC], BF16, tag="qv", name="qv", bufs=1)
        nc.vector.tensor_scalar(v[:, :], u2[:, :], -0.5, 1.5,
                                op0=ALU.mult, op1=ALU.add)
        y1 = pbw.tile([128, BC], BF16, tag="qy1", name="qy1", bufs=1)
        nc.vector.tensor_mul(y1[:, :], v[:, :], y0[:, :])
        xaw = pbw.tile([128, BC], BF16, tag="xaw", name="xaw", bufs=1)
        nc.vector.tensor_mul(xaw[:, :], hbw[:, :], y1[:, :])
        txa = pbw.tile([128, BC], BF16, tag="txa", name="txa", bufs=1)
        nc.scalar.activation(txa[:, :], xaw[:, :], AF.Tanh)
        nc.vector.tensor_copy(get_xbu(l0)[0:64, :], txa[0:64, :])
        nc.vector.tensor_copy(get_xbu(l1)[0:64, :], txa[64:128, :])

    def emit_bu_b_half(l, g):
        """One level-step of the message chain for batch-half g (cols
        [g*1024, g*1024+1024), packed [128,512]). The two halves run as
        independent chains one level apart, so each op has a full
        iteration of slack."""
        X = get_xbu(l)
        Xn = get_xbu(l - 1)
        c0 = g * HALF
        h2p = ppB.tile([128, 512], F32, tag=f"bA{g}", name="h2p", bufs=1)
        nc.tensor.matmul(h2p[0:64, :], uW2t[:, :], X[:, c0:c0 + 512])
        nc.tensor.matmul(h2p[64:128, :], uW2t[:, :], X[:, c0 + 512:c0 + 1024])
        h2s = pbh.tile([128, 512], BF16, tag=f"h2s{g}", name="h2s", bufs=1)
        nc.scalar.activation(h2s[:, :], h2p[:, :], AF.Tanh, bias=ub2d[:, 0:1])
        msgp = ppB.tile([128, 512], F32, tag=f"bB{g}", name="msgp", bufs=1)
        nc.tensor.matmul(msgp[:, :], uW3bd[:, :], h2s[:, :])
        sqm = pbh.tile([128, 512], BF16, tag=f"sqm{g}", name="sqm", bufs=1)
        nc.scalar.activation(sqm[:, :], msgp[:, :], AF.Square,
                             bias=ub3d[:, 0:1])
        nsb = ppB.tile([128, 512], F32, tag=f"bA{g}", name="nsb", bufs=1)
        nc.tensor.matmul(nsb[:, :], bdones[:, :], sqm[:, :])
        tq = pbh.tile([128, 512], F32, tag=f"qbt{g}", name="qbt", bufs=1)
        nc.vector.tensor_scalar(
            tq[:, :].bitcast(I32), nsb[:, :].bitcast(I32), 1, -1,
            op0=ALU.arith_shift_right, op1=ALU.bitwise_xor)
        y0q = pbh.tile([128, 512], F32, tag=f"qby0{g}", name="qby0", bufs=1)
        nc.vector.tensor_scalar_add(y0q[:, :].bitcast(I32),
                                    tq[:, :].bitcast(I32), MAGIC32)
        wq = pbh.tile([128, 512], F32, tag=f"qbw{g}", name="qbw", bufs=1)
        nc.scalar.activation(wq[:, :], y0q[:, :], AF.Square)
        uq = pbh.tile([128, 512], F32, tag=f"qbu{g}", name="qbu", bufs=1)
        nc.vector.scalar_tensor_tensor(uq[:, :], wq[:, :], -0.5, nsb[:, :],
                                       op0=ALU.mult, op1=ALU.mult)
        y1q = pbh.tile([128, 512], F32, tag=f"qby1{g}", name="qby1", bufs=1)
        nc.vector.scalar_tensor_tensor(y1q[:, :], uq[:, :], 1.5, y0q[:, :],
                                       op0=ALU.add, op1=ALU.mult)
        tms = pbh.tile([128, 512], BF16, tag=f"tms{g}", name="tms", bufs=1)
        nc.vector.scalar_tensor_tensor(tms[:, :], msgp[:, :], ub3d[:, 0:1],
                                       y1q[:, :], op0=ALU.add, op1=ALU.mult)
        nc.scalar.activation(Xn[64:128, c0:c0 + 512], tms[0:64, :], AF.Tanh)
        nc.scalar.activation(Xn[64:128, c0 + 512:c0 + 1024], tms[64:128, :],
                             AF.Tanh)

    # state view: [pair, partition(batch%128), block, 2*S contiguous values]
    st_pair = state.ap().rearrange("(k p) (lp w) v -> lp p k (w v)", p=128, w=2)

    with tc.tile_pool(name="xtpool", bufs=3) as pxt:

        def make_xt(p):
            xt = pxt.tile([128, BC], BF16, tag="xt", name=f"xt{p}")
            for kg in range(2):
                stg = pxt.tile([128, 8 * 2 * S], BF16, tag="stg", name="stg",
                               bufs=2)
                nc.gpsimd.dma_start(
                    stg[:, :].rearrange("q (k u) -> q k u", k=8),
                    st_pair[p][:, 8 * kg:8 * (kg + 1)])
                tp = ppB.tile([128, 1024], BF16, tag=f"bA{kg}", name="tp",
                              bufs=1)
                for ki in range(8):
                    nc.tensor.transpose(tp[:, ki * 128:(ki + 1) * 128],
                                        stg[:, ki * 128:(ki + 1) * 128],
                                        ident[:, :])
                nc.scalar.copy(xt[:, kg * 1024:(kg + 1) * 1024], tp[:, :])
            return xt

        xts = {NPAIR - 1: make_xt(NPAIR - 1), NPAIR - 2: make_xt(NPAIR - 2)}
        nc.gpsimd.memset(get_xbu(L - 1)[64:128, :], 0.0)  # tanh(m(20)) = 0

        def alloc_ab():
            hbw = pbw.tile([128, BC], BF16, tag="hbw", name="hbw", bufs=2)
            sqbw = pbw.tile([128, BC], BF16, tag="sqbw", name="sqbw", bufs=2)
            return hbw, sqbw

        hs = alloc_ab()
        for g in range(2):
            emit_bu_a_front(NPAIR - 1, xts, g, *hs)
        emit_bu_a_back(NPAIR - 1, *hs)
        del xts[NPAIR - 1]
        # chain g=0 runs levels 19..0; chain g=1 lags one level behind.
        hs_pend = None
        for l0 in range(L - 1, -1, -1):
            if l0 + 1 <= L - 1:
                emit_bu_b_half(l0 + 1, 1)
            if l0 % 2 == 1:
                pf = (l0 - 1) // 2 - 1
                if pf >= 0:
                    hs_pend = alloc_ab()
                    for g in range(2):
                        emit_bu_a_front(pf, xts, g, *hs_pend)
                if pf >= 1:
                    xts[pf - 1] = make_xt(pf - 1)
            emit_bu_b_half(l0, 0)
            if l0 % 2 == 0 and l0 >= 2:
                p = l0 // 2 - 1
                emit_bu_a_back(p, *hs_pend)
                del xts[p]
        emit_bu_b_half(0, 1)

    bu_ctx.close()

    # ---------------- TD phase ----------------
    td_ctx = contextlib.ExitStack()
    ptd = td_ctx.enter_context(tc.tile_pool(name="tdwork", bufs=2))
    pmd = td_ctx.enter_context(tc.tile_pool(name="mdwork", bufs=2))
    ppL = {}
    for mlp in "am":
        for mh in range(2):
            ppL[(mlp, mh)] = td_ctx.enter_context(
                tc.tile_pool(name=f"psL{mlp}{mh}", bufs=1, space="PSUM"))
    ppMD = td_ctx.enter_context(tc.tile_pool(name="psMD", bufs=1, space="PSUM"))
    ppNS = td_ctx.enter_context(tc.tile_pool(name="psNS", bufs=1, space="PSUM"))
    ppT = td_ctx.enter_context(tc.tile_pool(name="psT", bufs=1, space="PSUM"))

    nc.gpsimd.memset(get_xbu(-1)[0:64, :], 0.0)  # tanh(md(-1)) = 0

    def relu(ps, hs, bias, on_act):
        if on_act:
            nc.scalar.activation(hs[:, :], ps[:, :], AF.Relu, bias=bias[:, 0:1])
        else:
            nc.vector.tensor_scalar(hs[:, :], ps[:, :], bias[:, 0:1], 0.0,
                                    op0=ALU.add, op1=ALU.max)

    for l in range(L):
        X = get_xbu(l - 1)
        last = (l == L - 1)
        aps = ppT.tile([97, 512], F32, tag="aps")
        h1a = {}
        h2a = {}
        # pass 1 per chunk: W1 (a+m) + relus, W2m + relus, mW3 -> packed mdg
        # mdg [128,1024]: even chunks on partitions 0:64, odd on 64:128;
        # chunk pair (2h, 2h+1) in columns h*512:(h+1)*512.
        if not last:
            mdg = ppMD.tile([128, 1024], F32, tag="md", name="mdg")
        mdbw = {}
        sqd = {}
        for c in range(NCH):
            cc = slice(c * CH, (c + 1) * CH)
            mlps = "a" if last else "am"
            h1 = {}
            for mlp in mlps:
                W1p = aW1p if mlp == "a" else mW1p
                for mh in range(2):
                    ps = ppL[(mlp, mh)].tile([128, CH], F32, tag=f"L{mlp}{mh}",
                                             name=f"L{mlp}{mh}")
                    nc.tensor.matmul(ps[:, :], W1p[:, mh * 128:(mh + 1) * 128],
                                     X[:, cc])
                    tag = f"h1a{mh}c{c}" if mlp == "a" else f"h1m{mh}"
                    hs = ptd.tile([128, CH], BF16, tag=tag, name=tag)
                    relu(ps, hs, (ab1t if mlp == "a" else mb1t)[mh],
                         on_act=(mlp == "a"))
                    h1[(mlp, mh)] = hs
            h1a[c] = (h1[("a", 0)], h1[("a", 1)])
            if last:
                continue
            # W2m + relu + mW3 (the latency-critical md branch)
            h2m = {}
            for mh in range(2):
                ps = ppL[("m", mh)].tile([128, CH], F32, tag=f"Lm{mh}",
                                         name=f"L2m{mh}")
                ms_ = slice(mh * 128, (mh + 1) * 128)
                nc.tensor.matmul(ps[:, :], mW2k[0][:, ms_], h1[("m", 0)][:, :],
                                 start=True, stop=False)
                nc.tensor.matmul(ps[:, :], mW2k[1][:, ms_], h1[("m", 1)][:, :],
                                 start=False, stop=True)
                hs = ptd.tile([128, CH], BF16, tag=f"h2m{mh}", name=f"h2m{mh}")
                relu(ps, hs, mb2t[mh], on_act=False)
                h2m[mh] = hs
            pp = slice(64 * (c % 2), 64 * (c % 2) + 64)
            ccol = slice((c // 2) * 512, (c // 2) * 512 + 512)
            tpos = (0, 64 * (c % 2))
            nc.tensor.matmul(mdg[pp, ccol], mW3k[0][:, :], h2m[0][:, :],
                             start=True, stop=False, tile_position=tpos)
            nc.tensor.matmul(mdg[pp, ccol], mW3k[1][:, :], h2m[1][:, :],
                             start=False, stop=True, tile_position=tpos)
            if c % 2 == 1:
                # per-half bias-add + square as soon as this half's mW3 is
                # done, so md_half(0) need not wait for chunks 2 and 3.
                h = c // 2
                mdbw[h] = pmd.tile([128, 512], BF16, tag=f"mdbw{h}",
                                   name="mdbw")
                nc.scalar.activation(mdbw[h][:, :], mdg[:, ccol], AF.Identity,
                                     bias=mb3d[:, 0:1])
                sqd[h] = pmd.tile([128, 512], BF16, tag=f"sqd{h}", name="sqd")
                nc.scalar.activation(sqd[h][:, :], mdg[:, ccol], AF.Square,
                                     bias=mb3d[:, 0:1])
        # W2a + relu: PE fill work at the level join, split around the
        # nsq matmuls so the tensor engine stays fed during the quake tail.
        def w2a_chunk(c):
            for mh in range(2):
                ps = ppL[("a", mh)].tile([128, CH], F32, tag=f"La{mh}",
                                         name=f"L2a{mh}")
                ms_ = slice(mh * 128, (mh + 1) * 128)
                nc.tensor.matmul(ps[:, :], aW2k[0][:, ms_],
                                 h1a[c][0][:, :], start=True, stop=False)
                nc.tensor.matmul(ps[:, :], aW2k[1][:, ms_],
                                 h1a[c][1][:, :], start=False, stop=True)
                hs = ptd.tile([128, CH], BF16, tag=f"h2a{mh}c{c}",
                              name=f"h2a{mh}c{c}")
                relu(ps, hs, ab2t[mh], on_act=True)
                h2a[c] = h2a.get(c, {})
                h2a[c][mh] = hs

        Xn1 = get_xbu(l)

        def md_half(h):
            nsq = ppNS.tile([128, 512], F32, tag="nsq", name="nsq")
            nc.tensor.matmul(nsq[:, :], bdones[:, :], sqd[h][:, :])
            y1 = quake32(pmd, nsq[:, :], 512, f"qd{h}")
            tms = pmd.tile([128, 512], BF16, tag=f"tmsd{h}", name="tmsd")
            nc.vector.tensor_mul(tms[:, :], mdbw[h][:, :], y1[:, :])
            ce, co = 2 * h * CH, (2 * h + 1) * CH
            nc.scalar.activation(Xn1[0:64, ce:ce + CH], tms[0:64, :],
                                 AF.Tanh)
            nc.scalar.activation(Xn1[0:64, co:co + CH], tms[64:128, :],
                                 AF.Tanh)

        if not last:
            md_half(0)
        w2a_chunk(0)
        w2a_chunk(1)
        if not last:
            md_half(1)
        w2a_chunk(2)
        w2a_chunk(3)
        # aW3 (more PE fill), then store the action row
        for c in range(NCH):
            nc.tensor.matmul(aps[32 * c:32 * c + 1, :], aW3k[0][:, :],
                             h2a[c][0][:, :], start=True, stop=False,
                             tile_position=(0, 32 * c))
            nc.tensor.matmul(aps[32 * c:32 * c + 1, :], aW3k[1][:, :],
                             h2a[c][1][:, :], start=False, stop=True,
                             tile_position=(0, 32 * c))
        asb = ptd.tile([97, 512], F32, tag="asb")
        nc.vector.tensor_copy(asb[:, :], aps[:, :])
        nc.scalar.dma_start(a_store[l:l + 1, :], asb[0:97:32, :])

    td_ctx.close()

    # ---------------- output: tanh, transpose, DMA ----------------
    pout = ctx.enter_context(tc.tile_pool(name="outp", bufs=1))
    att = pout.tile([32, BC], F32, tag="att")
    nc.gpsimd.memset(att[:, :], 0.0)
    nc.scalar.activation(att[0:20, :], a_store[0:20, :], AF.Tanh,
                         bias=ab3t[0:20, 0:1])
    otr = pout.tile([32, BC], F32, tag="otr")
    for k in range(NBLK):
        nc.vector.transpose(otr[:, k * 128:(k + 1) * 128],
                            att[:, k * 128:(k + 1) * 128])
    # otr[r, k*128 + 32*bj + c] = action(batch k*128 + 32*bj + r, level c)
    dst = out_ext.ap().rearrange("(k bj r) l -> r k bj l", r=32, bj=4)
    src = otr[:, :].rearrange("r (k bj c) -> r k bj c", bj=4, c=32)[:, :, :, 0:20]
    nc.sync.dma_start(dst, src)

    ctx.close()


_NC_CACHE = None


def _get_nc():
    global _NC_CACHE
    if _NC_CACHE is None:
        nc = bacc.Bacc("TRN2", target_bir_lowering=False, debug=False)
        _build(nc)
        nc.compile()
        _NC_CACHE = nc
    return _NC_CACHE


def kernel(**inputs) -> np.ndarray:
    nc = _get_nc()
    state = inputs["state"]
    in_maps = []
    for i in range(N_CORES):
        m = {"state": np.ascontiguousarray(state[i * BC:(i + 1) * BC])}
        for n in WNAMES:
            m[n] = np.ascontiguousarray(inputs[n])
        in_maps.append(m)
    res = run_bass_kernel_spmd(nc, in_maps, core_ids=list(range(N_CORES)))
    return np.concatenate([res.results[i]["out"] for i in range(N_CORES)], axis=0)
